# revision 33
# baseline (speedup 1.0000x reference)
"""Distributed Trainium2 kernel for nn_Attention_2654289789382 (sparse_attention).

Math (reference):
    sigma = sigmoid(x @ W_sigma + b_sigma)           (b, h, n)
    den_i = exp(sigma)+1 ;  r_i = 1/den_i = sigmoid(-sigma)   in (0.2689, 0.5)
    prior[i,j] = softmax_j(-|i-j| * r_i)
    out = (prior @ v) reshaped @ W_out + b_out,  v = x @ W_v

Structure exploited:
  * r_i >= 0.2689  =>  banded attention, band half-width 64. Per 128-row
    i-block only 2 j-tiles of 128 (at +-64) contribute.
  * softmax denominator in closed form (two-sided geometric series):
        den_i = 1 + (2z - z^(i+1) - z^(n-i)) / (1-z),  z = exp(-r_i)
    Edge corrections z^(i+1)/z^(n-i) computed for boundary blocks only.
  * Q[j,i] = exp(|i-j| * -r_i) in matmul-rhs layout via stride-0 broadcast
    DMA of -r (DRAM hop), a DVE multiply against a |dist| master, and one
    ScalarE Exp per (chunk, head).
  * AV bf16, two head-pairs per [128,1024] psum super-tile; 1/den fused
    into the psum->SBUF norm; out^T lands in W_out's lhsT layout.
  * Projection accumulated per head-pair as norms complete (partial
    accumulation chains); bias add + bf16 cast; 4 output DMAs.

Scheduling (v4):
  * ScalarE uses ONLY Exp + Copy (sigmoids as exp+reciprocal on DVE):
    act-table switches cost 1.3us each, so one table loads once.
  * b_sigma folded into the sigma matmul as a rank-1 (ones x bsig) row so
    ScalarE's first Exp reads the sigma PSUM directly - no DVE hop.
  * Inputs on the two HWDGE queues only (gpsimd SWDGE DMAs start ~3us
    late); xpk split 2+2 sync/scalar, Ws in a tiny early tensor.
  * All 16 ARG multiplies on DVE (GpSimd/DVE SBUF contention sinks both);
    V psum evacuations: V0-4 on ScalarE pre-Exp, V5-8 on DVE post-ARGs,
    so the DVE queue never blocks the Exp pacing chain.

Sharding: 8 cores = 4 batches x 2 sequence halves; no collectives.
"""

import numpy as np
import ml_dtypes

import concourse.bass as bass
import concourse.mybir as mybir
import concourse.tile as tile
from concourse import bacc
from concourse.bass_utils import run_bass_kernel_spmd

F32 = mybir.dt.float32
BF16 = mybir.dt.bfloat16

B, N, D = 4, 2048, 512
H, DH = 8, 64
HALF = N // 2            # 1024 rows per core
NBLK = HALF // 128       # 8 i-blocks per core
NVT = 9                  # V tiles at odd 64-offsets
CB = 4                   # i-blocks per ARG/exp chunk
XC = 1152                # used x^T cols per dt tile (padded cols 64..1216)
# wpk: Wv (4dt x 512) | Wo (4dt x 512) | m2r 256 | identb 128 | boutb 512
WPK_COLS = 2048 + 2048 + 256 + 128 + 512
# wsb: Ws (4dt x 8) | row0: bsig 8 | row0: ones 128
WSB_COLS = 32 + 8 + 128

_nc_cache = None


def _build_nc():
    nc = bacc.Bacc("TRN2", target_bir_lowering=False, debug=False)

    xpk = nc.dram_tensor("xpk", [128, 4 * XC], BF16, kind="ExternalInput")
    wsb = nc.dram_tensor("wsb", [128, WSB_COLS], BF16, kind="ExternalInput")
    wpk = nc.dram_tensor("wpk", [128, WPK_COLS], BF16, kind="ExternalInput")
    fpk = nc.dram_tensor("fpk", [128, 24], F32, kind="ExternalInput")
    out = nc.dram_tensor("out", [HALF, D], BF16, kind="ExternalOutput")
    negr_d = nc.dram_tensor("negr_d", [4, 2048], BF16)   # rows = head pairs
    inv_d = nc.dram_tensor("inv_d", [2, 4096], BF16)     # rows = even/odd

    EXP = mybir.ActivationFunctionType.Exp
    COPY = mybir.ActivationFunctionType.Copy
    MUL = mybir.AluOpType.mult
    ADD = mybir.AluOpType.add

    with tile.TileContext(nc) as tc:
        with (
            tc.tile_pool(name="const", bufs=1) as cpool,
            tc.tile_pool(name="vpool", bufs=1) as vpool,
            tc.tile_pool(name="otpool", bufs=1) as otpool,
            tc.tile_pool(name="sg", bufs=1) as sgpool,
            tc.tile_pool(name="bc", bufs=1) as bcpool,
            tc.tile_pool(name="ap", bufs=6) as apool,
            tc.tile_pool(name="qp", bufs=16) as qpool,
            tc.tile_pool(name="fin", bufs=2) as fpool,
        ):
            # ---------------- input DMAs (HWDGE queues only) -------------
            wsb_t = cpool.tile([128, WSB_COLS], BF16, tag="wsb")
            nc.sync.dma_start(wsb_t[:], wsb[:, :])
            fpk_t = cpool.tile([128, 24], F32, tag="fpk")
            nc.scalar.dma_start(fpk_t[:], fpk[:, :])
            xpk_t = cpool.tile([128, 4 * XC], BF16, tag="xpk")
            for dt, eng in enumerate((nc.sync, nc.scalar, nc.sync,
                                      nc.scalar)):
                eng.dma_start(xpk_t[:, dt * XC:(dt + 1) * XC],
                              xpk[:, dt * XC:(dt + 1) * XC])
            wpk_t = cpool.tile([128, WPK_COLS], BF16, tag="wpk")
            hw = WPK_COLS // 2
            nc.sync.dma_start(wpk_t[:, 0:hw], wpk[:, 0:hw])
            nc.scalar.dma_start(wpk_t[:, hw:], wpk[:, hw:])

            def xv(dt, c0, w):      # x^T slice at padded col c0
                return xpk_t[:, dt * XC + (c0 - 64):dt * XC + (c0 - 64) + w]

            def wv_v(dt):
                return wpk_t[:, dt * 512:(dt + 1) * 512]

            def wv_o(dt):
                return wpk_t[:, 2048 + dt * 512:2048 + (dt + 1) * 512]

            def wv_s(dt):
                return wsb_t[:, dt * 8:(dt + 1) * 8]

            bsig_row = wsb_t[0:1, 32:40]
            ones_row = wsb_t[0:1, 40:168]
            m2r_t = wpk_t[:, 4096:4352]
            identb_t = wpk_t[:, 4352:4480]
            boutb_t = wpk_t[:, 4480:4992]
            ivp1_t = fpk_t[:, 8:16]     # block 0 only, h-replicated
            ivnm_t = fpk_t[:, 16:24]    # block 7 only, h-replicated

            # warmup: preload the (only) Exp act table during input DMAs
            warm = sgpool.tile([128, 1], F32, tag="warm")
            nc.vector.memset(warm[:], 0.0)
            warm2 = sgpool.tile([128, 1], F32, tag="warm2")
            nc.scalar.activation(warm2[:], warm[:], EXP)

            # V tiles (single, [128, 512])
            Vt = [vpool.tile([128, 512], BF16, name=f"V{k}", tag=f"V{k}")
                  for k in range(NVT)]

            R_all = bcpool.tile([128, H * HALF], BF16, tag="R_all")
            Iv_pair = bcpool.tile([128, 4 * HALF], BF16, tag="Iv_pair")
            outT = otpool.tile([128, 4 * HALF], BF16, tag="outT")

            def emit_arg(ch, hp, hh):
                # ARG[j, (b,o,q)] = |dist| * -r  for head 2hp+hh  (DVE)
                R = R_all[:, hp * 2048 + hh * 1024 + ch * CB * 128:
                          hp * 2048 + hh * 1024 + (ch + 1) * CB * 128]
                ARG = apool.tile([128, CB * 256], BF16, tag="ARG", name="ARG")
                nc.vector.tensor_tensor(
                    ARG[:].rearrange("p (b o q) -> p b o q", b=CB, o=2),
                    m2r_t
                    .rearrange("p (one o q) -> p one o q", one=1, o=2)
                    .broadcast_to((128, CB, 2, 128)),
                    R.rearrange("p (b one q) -> p b one q", b=CB, one=1)
                    .broadcast_to((128, CB, 2, 128)),
                    op=MUL,
                )
                return ARG

            def emit_exp(ARG):
                Q = qpool.tile([128, CB * 256], BF16, tag="Q", name="Q")
                nc.scalar.activation(Q[:], ARG[:], EXP)
                return Q

            # ============ phase 1: sigma, V, den, ARG/EXP ============
            with (
                tc.tile_pool(name="pss", bufs=1, space="PSUM") as pss,
                tc.tile_pool(name="psv", bufs=3, space="PSUM") as psv,
            ):
                # sigma matmuls, bias folded in as rank-1 (ones x bsig)
                ps_sig = pss.tile([128, NBLK * H], F32, tag="ps_sig")
                for b in range(NBLK):
                    for dt in range(4):
                        nc.tensor.matmul(
                            ps_sig[:, b * H:(b + 1) * H],
                            lhsT=xv(dt, 128 + b * 128, 128),
                            rhs=wv_s(dt),
                            start=(dt == 0),
                            stop=False,
                        )
                    nc.tensor.matmul(
                        ps_sig[:, b * H:(b + 1) * H],
                        lhsT=ones_row,
                        rhs=bsig_row,
                        start=False,
                        stop=True,
                    )

                # sigma chain, Exp-only (reads psum directly):
                #   sig = 1/(1+exp(-s));  r = 1/(1+exp(sig));  negr = -r
                e1 = sgpool.tile([128, NBLK * H], F32, tag="e1")
                nc.scalar.activation(e1[:], ps_sig[:], EXP, scale=-1.0)
                d1 = sgpool.tile([128, NBLK * H], F32, tag="d1")
                nc.vector.tensor_scalar_add(d1[:], e1[:], 1.0)
                sig = sgpool.tile([128, NBLK * H], F32, tag="sig")
                nc.vector.reciprocal(sig[:], d1[:])
                e2 = sgpool.tile([128, NBLK * H], F32, tag="e2")
                nc.scalar.activation(e2[:], sig[:], EXP)
                d2 = sgpool.tile([128, NBLK * H], F32, tag="d2")
                nc.vector.tensor_scalar_add(d2[:], e2[:], 1.0)
                r_all = sgpool.tile([128, NBLK * H], F32, tag="r_all")
                nc.vector.reciprocal(r_all[:], d2[:])
                negr_b = sgpool.tile([128, NBLK * H], BF16, tag="negr_b")
                nc.vector.tensor_scalar(
                    negr_b[:].rearrange("p (h b) -> p h b", b=NBLK),
                    r_all[:].rearrange("p (b h) -> p h b", h=H),
                    -1.0, None, MUL,
                )
                ptn = pss.tile([64, 128], BF16, tag="pt", name="ptn")
                nc.tensor.transpose(ptn[:], negr_b[:], identb_t)
                negrT = sgpool.tile([64, 128], BF16, tag="negrT")
                nc.vector.tensor_copy(negrT[:], ptn[:])
                nc.sync.dma_start(
                    negr_d.ap().rearrange("hp (two b p) -> (hp two b) p",
                                          two=2, b=NBLK),
                    negrT[:, :],
                )
                for hp in range(4):
                    nc.sync.dma_start(
                        R_all[:, hp * 2048:(hp + 1) * 2048],
                        negr_d[hp:hp + 1, :].to_broadcast((128, 2048)),
                    )

                # V matmuls (single tiles)
                pvs = [None] * NVT

                def emit_v_mm(k):
                    pv = psv.tile([128, 512], F32, tag="pv", name="pv")
                    for dt in range(4):
                        nc.tensor.matmul(
                            pv[:],
                            lhsT=xv(dt, 64 + 128 * k, 128),
                            rhs=wv_v(dt),
                            start=(dt == 0),
                            stop=(dt == 3),
                        )
                    pvs[k] = pv

                def evac_v(k, eng):
                    if eng is nc.scalar:
                        nc.scalar.activation(Vt[k][:], pvs[k][:], COPY)
                    else:
                        nc.vector.tensor_copy(Vt[k][:], pvs[k][:])

                for k in range(NVT):
                    emit_v_mm(k)
                    if k <= 4:
                        evac_v(k, nc.scalar)

                # 1/den closed form (slack path, DVE)
                z = sgpool.tile([128, NBLK * H], F32, tag="z")
                nc.scalar.activation(z[:], r_all[:], EXP, scale=-1.0)
                argA = sgpool.tile([128, H], F32, tag="argA")
                nc.vector.tensor_mul(argA[:], r_all[:, 0:H], ivp1_t)
                expA = sgpool.tile([128, H], F32, tag="expA")
                nc.scalar.activation(expA[:], argA[:], EXP)
                argB = sgpool.tile([128, H], F32, tag="argB")
                nc.vector.tensor_mul(argB[:], r_all[:, 56:64], ivnm_t)
                expB = sgpool.tile([128, H], F32, tag="expB")
                nc.scalar.activation(expB[:], argB[:], EXP)
                w = sgpool.tile([128, NBLK * H], F32, tag="w")
                nc.vector.tensor_scalar(w[:], z[:], -1.0, 1.0, MUL, ADD)
                t1 = sgpool.tile([128, NBLK * H], F32, tag="t1")
                nc.vector.tensor_scalar_mul(t1[:], z[:], 2.0)
                nc.vector.tensor_sub(t1[:, 0:H], t1[:, 0:H], expA[:])
                nc.vector.tensor_sub(t1[:, 56:64], t1[:, 56:64], expB[:])
                u = sgpool.tile([128, NBLK * H], F32, tag="u")
                nc.vector.tensor_add(u[:], w[:], t1[:])
                ru = sgpool.tile([128, NBLK * H], F32, tag="ru")
                nc.vector.reciprocal(ru[:], u[:])
                inv_c = sgpool.tile([128, NBLK * H], F32, tag="inv_c")
                nc.vector.tensor_mul(inv_c[:], w[:], ru[:])

                # ch0 ARG/EXP in consumption order
                Q0 = [[None, None] for _ in range(4)]
                for hp in range(4):
                    a0 = emit_arg(0, hp, 0)
                    a1 = emit_arg(0, hp, 1)
                    Q0[hp][0] = emit_exp(a0)
                    Q0[hp][1] = emit_exp(a1)

                # inv transpose chain (Iv needed only at the first norm)
                inv_b = sgpool.tile([128, NBLK * H], BF16, tag="inv_b")
                nc.vector.tensor_copy(
                    inv_b[:].rearrange("p (two hp b) -> p two hp b",
                                       two=2, hp=4),
                    inv_c[:].rearrange("p (b hp two) -> p two hp b",
                                       hp=4, two=2),
                )
                pti = pss.tile([64, 128], BF16, tag="pt", name="pti")
                nc.tensor.transpose(pti[:], inv_b[:], identb_t)
                invT = sgpool.tile([64, 128], BF16, tag="invT")
                nc.vector.tensor_copy(invT[:], pti[:])
                nc.sync.dma_start(
                    inv_d.ap().rearrange("two (hp b p) -> (two hp b) p",
                                         hp=4, b=NBLK),
                    invT[:, :],
                )
                for e in range(2):
                    nc.sync.dma_start(
                        Iv_pair[e * 64:(e + 1) * 64, :],
                        inv_d[e:e + 1, :].to_broadcast((64, 4096)),
                    )

                # ch1 ARGs ahead of need + remaining V evacuations
                A1 = {}
                for hp in range(4):
                    A1[(hp, 0)] = emit_arg(1, hp, 0)
                    A1[(hp, 1)] = emit_arg(1, hp, 1)
                    evac_v(5 + hp, nc.vector)

                # ch1 EXPs in consumption order
                Q1 = [[None, None] for _ in range(4)]
                for hp in range(4):
                    Q1[hp][0] = emit_exp(A1[(hp, 0)])
                    Q1[hp][1] = emit_exp(A1[(hp, 1)])

            # ============ phase 2: AV, norm, proj, out ============
            with (
                tc.tile_pool(name="psa", bufs=2, space="PSUM") as psa,
                tc.tile_pool(name="psf", bufs=1, space="PSUM") as psf,
            ):
                def emit_av(ch, hpp, Qpair):
                    pav = psa.tile([128, 1024], F32, tag="pav", name="pav")
                    for u_ in range(2):
                        hp = 2 * hpp + u_
                        for bi in range(CB):
                            b = ch * CB + bi
                            for hh in range(2):
                                h = 2 * hp + hh
                                for o in range(2):
                                    nc.tensor.matmul(
                                        pav[hh * 64:(hh + 1) * 64,
                                            u_ * 512 + bi * 128:
                                            u_ * 512 + (bi + 1) * 128],
                                        lhsT=Vt[b + o][:, h * 64:
                                                       (h + 1) * 64],
                                        rhs=Qpair[u_][hh][:,
                                            bi * 256 + o * 128:
                                            bi * 256 + (o + 1) * 128],
                                        start=(o == 0),
                                        stop=(o == 1),
                                    )
                    return pav

                def emit_norm(ch, hpp, pav):
                    ot = outT[:].rearrange("p (hp i) -> p hp i", hp=4)[
                        :, 2 * hpp:2 * hpp + 2,
                        ch * CB * 128:(ch + 1) * CB * 128]
                    iv = Iv_pair[:].rearrange("p (hp i) -> p hp i", hp=4)[
                        :, 2 * hpp:2 * hpp + 2,
                        ch * CB * 128:(ch + 1) * CB * 128]
                    nc.vector.tensor_mul(
                        ot, pav[:].rearrange("p (u i) -> p u i", u=2), iv)

                def emit_proj(ch, hp, pfs):
                    for bi in range(CB):
                        b = ch * CB + bi
                        nc.tensor.matmul(
                            pfs[bi // 2][:, (bi % 2) * 512:
                                         (bi % 2) * 512 + 512],
                            lhsT=outT[:, hp * HALF + b * 128:
                                      hp * HALF + (b + 1) * 128],
                            rhs=wv_o(hp),
                            start=(hp == 0),
                            stop=(hp == 3),
                        )

                def emit_fin(ch, pfs):
                    for half in range(2):
                        fin = fpool.tile([128, 1024], BF16, tag="fin",
                                         name="fin")
                        nc.vector.tensor_tensor(
                            fin[:].rearrange("p (blk d) -> p blk d", blk=2),
                            pfs[half][:].rearrange("p (blk d) -> p blk d",
                                                   blk=2),
                            boutb_t.rearrange("p (one d) -> p one d", one=1)
                            .broadcast_to((128, 2, 512)),
                            op=ADD,
                        )
                        b0 = ch * CB + half * 2
                        nc.sync.dma_start(
                            out[b0 * 128:(b0 + 2) * 128, :]
                            .rearrange("(blk p) d -> p blk d", blk=2),
                            fin[:].rearrange("p (blk d) -> p blk d", blk=2),
                        )

                for ch, Qc in ((0, Q0), (1, Q1)):
                    pfs = [psf.tile([128, 1024], F32, name=f"pf{u}",
                                    tag=f"pf{u}") for u in range(2)]
                    for hpp in range(2):
                        pav = emit_av(ch, hpp, Qc[2 * hpp:2 * hpp + 2])
                        emit_norm(ch, hpp, pav)
                        emit_proj(ch, 2 * hpp, pfs)
                        emit_proj(ch, 2 * hpp + 1, pfs)
                    emit_fin(ch, pfs)

    nc.compile()
    return nc


def _make_in_maps(x, W_v, W_sigma, b_sigma, W_out, b_out):
    bf = ml_dtypes.bfloat16
    m2r1 = np.empty((128, 256), dtype=np.float32)
    p = np.arange(128, dtype=np.float32)[:, None]
    q = np.arange(128, dtype=np.float32)[None, :]
    for o in range(2):
        m2r1[:, o * 128:(o + 1) * 128] = np.abs(q - p + 64.0 - 128.0 * o)
    identb = np.eye(128, dtype=np.float32)

    wsb1 = np.zeros((128, WSB_COLS), dtype=np.float32)
    for dt in range(4):
        wsb1[:, dt * 8:(dt + 1) * 8] = W_sigma[dt * 128:(dt + 1) * 128]
    wsb1[0, 32:40] = b_sigma
    wsb1[0, 40:168] = 1.0
    wsb = np.ascontiguousarray(wsb1.astype(bf))

    wblocks = [W_v[dt * 128:(dt + 1) * 128] for dt in range(4)]
    wblocks += [W_out[dt * 128:(dt + 1) * 128] for dt in range(4)]
    wblocks += [m2r1, identb, np.broadcast_to(b_out[None, :], (128, D))]
    wpk = np.ascontiguousarray(np.concatenate(wblocks, axis=1).astype(bf))

    in_maps = []
    for c in range(8):
        bb, half = c // 2, c % 2
        i_start = half * HALF
        xp = np.zeros((HALF + 256, D), dtype=np.float32)
        j_lo = max(0, i_start - 128)
        j_hi = min(N, i_start + HALF + 128)
        xp[j_lo - (i_start - 128):j_hi - (i_start - 128)] = x[bb, j_lo:j_hi]
        xT = xp.T.astype(bf)                       # [512, 1280]
        xpk = np.ascontiguousarray(
            xT.reshape(4, 128, HALF + 256)[:, :, 64:64 + XC]
            .transpose(1, 0, 2).reshape(128, 4 * XC))

        pcol = np.arange(128, dtype=np.float32)[:, None]
        ia0 = i_start + pcol
        ia7 = i_start + 7 * 128 + pcol
        ivp1 = np.repeat(-(ia0 + 1.0), H, axis=1).astype(np.float32)
        ivnm = np.repeat(-(float(N) - ia7), H, axis=1).astype(np.float32)
        fpk = np.ascontiguousarray(np.concatenate(
            [np.zeros((128, 8), np.float32), ivp1, ivnm], axis=1))

        in_maps.append({"xpk": xpk, "wsb": wsb, "wpk": wpk, "fpk": fpk})
    return in_maps


def kernel(x, W_v, W_sigma, b_sigma, W_out, b_out):
    global _nc_cache
    x = np.asarray(x, dtype=np.float32)
    W_v = np.asarray(W_v, dtype=np.float32)
    W_sigma = np.asarray(W_sigma, dtype=np.float32)
    b_sigma = np.asarray(b_sigma, dtype=np.float32)
    W_out = np.asarray(W_out, dtype=np.float32)
    b_out = np.asarray(b_out, dtype=np.float32)

    if _nc_cache is None:
        _nc_cache = _build_nc()
    nc = _nc_cache

    in_maps = _make_in_maps(x, W_v, W_sigma, b_sigma, W_out, b_out)
    res = run_bass_kernel_spmd(nc, in_maps, core_ids=list(range(8)))

    out = np.empty((B, N, D), dtype=np.float32)
    for c in range(8):
        bb, half = c // 2, c % 2
        out[bb, half * HALF:(half + 1) * HALF, :] = np.asarray(
            res.results[c]["out"], dtype=np.float32)
    return out
